# revision 3
# baseline (speedup 1.0000x reference)
"""Trainium2 Bass kernel for per-sample dynamic 3x3 conv (periodic padding).

y[b,o,h,w] = sum_{c,i,j} x[b,c,(h+i-1)%H,(w+j-1)%W] * wgt[b, c*9+i*3+j, o] + bias[b,o]

Shapes: x [16,64,128,128] f32, wgt [16,576,64] f32, bias [16,64] f32.
Sharding: data-parallel over batch, 2 samples per core on 8 cores.

Compute scheme: 64x64 PE-array tiling. Every matmul is K=64 (C), M=64 (O);
four quadrant matmuls run concurrently on the 128x128 array (full useful-MAC
rate: measured 216ns per 512-column tap block = 1 col/cycle/quadrant at
2.37GHz). Quadrant (s,g): sample s on array rows 64s:64s+64, col-group g
computes output rows 4g:4g+4 of an 8-row spatial tile; each quadrant owns a
private (partition x bank) PSUM region so all four accumulation chains
start/stop independently.

The wrap padding is materialized HOST-side: the kernel receives a
[2, 64, 130, 130] bf16 image with the periodic border baked in, so every
(tap, tile, quadrant) is a single clean N=512 matmul - no 1-wide column
slivers (which measured +26ns per affected tap block) and no boundary-tile
row splits. 16 tiles x 4 quadrants x 9 taps = 576 matmuls, all N=512.

Tiles are processed in PAIRS with the tap loop outer within a pair so
_dedup_ldweights drops the second tile's redundant InstLdweights (bf16
weights stay resident in the quadrant; LDWs background-load during the
previous tap's stream and are fully hidden).

Head: the first real matmul can only start once weights+first rows land
(~3 fixed us of DMA issue+DGE+sem latency), and the PE HAM clock-gate needs
~3.4us of sustained PE activity before it releases 2.4GHz. Eight garbage
warm-up matmuls (on a zeroed SBUF scratch, into a PSUM buffer the pool
reuses later - safe because the PE queue is in-order) run during the DMA
head so the real stream starts at full clock. Loads are split across BOTH
HWDGE rings (sync + scalar) in fine row segments so early tiles' rows land
ahead of the compute wavefront. The ACT table preload is triggered by a
const-input activation with no DMA dependency.

Evacuation per sample: the g=s quadrant is partition-aligned with the
output slot -> DVE tensor_scalar_add(+bias); the g=1-s quadrant crosses
partitions -> ACT activation(Identity, +bias). Stores ride the otherwise
idle GpSimd SWDGE ring so they never queue behind input segments; the last
tile stores as two 4-row halves to shorten the final drain. Inputs/weights
are cast to bf16 host-side and the output is stored bf16 and upcast on the
host.
"""

import numpy as np

KH = KW = 3
B, C, O, H, W = 16, 64, 64, 128, 128
HP, WP = H + 2, W + 2  # host-padded image
N_CORES = 8
BPC = B // N_CORES  # samples per core
TILE_ROWS = 8  # output rows per spatial tile (4 per quadrant col-group)
QROWS = TILE_ROWS // 2  # rows per quadrant -> N = 4*128 = 512
N_TILES = H // TILE_ROWS
G = 2  # spatial tiles per tap-outer pair (LDW reuse span)
N_WARMUP_MM = 8  # HAM warm-up matmuls (~3.4us cold) hidden under the DMA head

TAPS = [(i, j) for i in range(KH) for j in range(KW)]

_CACHE = {}


def _patch_tile_drain():
    """This container's walrus rejects Drain instructions carrying more than
    one sem wait (setupSyncWait: Too many sync wait commands). Re-emit the
    TileContext exit drain's waits as individual wait_ge instructions."""
    import concourse.tile as tile
    from concourse.vector_clock import ScopedClock

    if getattr(tile.TileContext, "_drain_patch_applied", False):
        return

    def _drain_and_barrier(self, tick_clock, wait_clock):
        import concourse.mybir as mybir

        nc = self.nc
        nop = nc.sync.nop(nofuse=True)
        wait_clock.add_sem_waits(nop.ins, ScopedClock({None: tick_clock.global_clock}))
        waits = list(nop.ins.sync_info.on_wait)
        nop.ins.sync_info.on_wait.clear()
        assert self.sems is not None
        by_name = {}
        for h in self.sems.allocated().values():
            by_name[getattr(h, "name", None)] = h
        # Spread the final sem waits round-robin over all engine queues
        # (serial ~115ns/wait on one queue otherwise); the sem-only barrier
        # below restores the all-engines ordering.
        engs = [nc.sync, nc.vector, nc.scalar, nc.tensor, nc.gpsimd]
        for k, w in enumerate(waits):
            h = by_name.get(w.ant_name)
            assert h is not None, f"no sem handle for {w.ant_name}"
            engs[k % len(engs)].wait_ge(h, w.wait_value)
        # per-engine drains except GpSimd's expensive dge_drain, then a
        # sem-only barrier (mirrors BassBlock.no_gpsimd_drain)
        gpsimd_type = nc.gpsimd.engine
        for eng_type, eng in nc.engines.items():
            if eng_type == gpsimd_type:
                continue
            d = mybir.InstDrain(
                name=nc.get_next_instruction_name(),
                ins=[],
                outs=[],
                bass_is_fusable=False,
            )
            d.engine = eng_type
            eng.add_instruction(d)
        nc.all_engine_barrier(sem_only=True)
        popped = nc._tile_sem_poison_stack.pop()
        assert popped is self._sem_poison
        # Skip the exit-time gpsimd dma_reset+sem_clear: the kernel preamble
        # already clears the whole kernel sem range on every execution, and
        # the gpsimd dge drain it implies costs microseconds. Only free the
        # IDs python-side (no further tiles are built after this point).
        sems = list(self.sems.allocated().values())
        sem_nums = [s.num for s in sems]
        nc._state.prepend_free_semaphores(sem_nums)
        for poison_set in nc._tile_sem_poison_stack:
            poison_set.update(sem_nums)

    tile.TileContext._drain_and_barrier = _drain_and_barrier
    tile.TileContext._drain_patch_applied = True


def _split_multi_waits(nc, max_waits=1):
    """Same walrus limitation, general form: any instruction carrying more
    than one sem wait fails setupSyncWait. Hoist excess waits onto dedicated
    single-wait NOPs on the same engine, placed just before the instruction."""
    import concourse.mybir as mybir

    for f in nc.m.functions:
        for blk in f.blocks:
            out = []
            changed = False
            for inst in blk.instructions:
                si = getattr(inst, "sync_info", None)
                waits = list(si.on_wait) if si is not None else []
                if len(waits) > max_waits:
                    changed = True
                    for w in waits[:-max_waits]:
                        out.append(
                            mybir.InstNoOp(
                                name=nc.get_next_instruction_name(),
                                engine=inst.engine,
                                sync_info=mybir.SyncInfo(on_wait=[w], on_update=[]),
                                bass_nofuse=True,
                            )
                        )
                    si.on_wait.clear()
                    for w in waits[-max_waits:]:
                        si.on_wait.append(w)
                out.append(inst)
            if changed:
                blk.instructions = out


def _dedup_ldweights(nc):
    """Drop InstLdweights that reload the exact weights already resident in
    the same PE array quadrant (bf16 weights persist across matmuls; a
    non-self-loading InstMatmult then reuses them - the pattern
    nc.tensor.ldweights documents as supported for 16-bit dtypes). Sync
    carried by a dropped load moves onto the next kept PE instruction."""
    import concourse.mybir as mybir

    def key(inst):
        a = inst.ins[0]
        return (
            a.memref,
            a.offset,
            tuple(tuple(d) for d in a.ap),
            tuple(inst.tile_position or (0, 0)),
            inst.perf_mode,
            inst.is_transpose,
        )

    dropped = 0
    for f in nc.m.functions:
        for blk in f.blocks:
            last = {}
            out = []
            pend_waits = []
            pend_updates = []
            for inst in blk.instructions:
                if getattr(inst, "engine", None) != mybir.EngineType.PE:
                    out.append(inst)
                    continue
                if isinstance(inst, mybir.InstLdweights):
                    pos = tuple(inst.tile_position or (0, 0))
                    k = key(inst)
                    if last.get(pos) == k:
                        si = inst.sync_info
                        if si is not None:
                            pend_waits.extend(si.on_wait)
                            pend_updates.extend(si.on_update)
                        dropped += 1
                        continue
                    last[pos] = k
                elif not isinstance(inst, mybir.InstMatmult):
                    last = {}  # unknown PE inst: conservatively forget
                if pend_waits or pend_updates:
                    if inst.sync_info is None:
                        inst.sync_info = mybir.SyncInfo(on_wait=[], on_update=[])
                    for w in pend_waits:
                        inst.sync_info.on_wait.append(w)
                    for u in pend_updates:
                        inst.sync_info.on_update.append(u)
                    pend_waits, pend_updates = [], []
                out.append(inst)
            assert not pend_waits and not pend_updates
            blk.instructions = out
    return dropped


def _build_module():
    import concourse.bass as bass
    import concourse.mybir as mybir
    import concourse.tile as tile

    _patch_tile_drain()

    f32 = mybir.dt.float32
    bf16 = mybir.dt.bfloat16

    nc = bass.Bass()
    x_d = nc.dram_tensor("input", [BPC, C, HP, WP], bf16, kind="ExternalInput")
    # weights pre-transposed host-side: wts[64*b+c, tap, o]
    w_d = nc.dram_tensor("wts", [128, KH * KW, O], bf16, kind="ExternalInput")
    b_d = nc.dram_tensor("bias", [BPC, O], f32, kind="ExternalInput")
    y_d = nc.dram_tensor("out", [BPC, O, H, W], bf16, kind="ExternalOutput")

    with tile.TileContext(nc) as tc:
        from contextlib import ExitStack

        ctx = ExitStack()
        with ctx:
            persist = ctx.enter_context(tc.tile_pool(name="persist", bufs=1))
            psum = ctx.enter_context(tc.tile_pool(name="psum", bufs=2, space="PSUM"))
            ostage = ctx.enter_context(tc.tile_pool(name="ostage", bufs=2))

            raw = persist.tile([128, HP, WP], bf16)
            x_bc = x_d.rearrange("b c h w -> (b c) h w")
            wts = persist.tile([128, KH * KW, O], bf16)
            bias_sb = persist.tile([128, 1], f32)
            warm = persist.tile([128, 512], bf16)
            scratch1 = persist.tile([128, 1], f32)

            # --- HAM warm-up + ACT table preload, no DMA dependencies.
            const0 = nc.const_aps.aps[(f32, 0.0)]
            nc.gpsimd.memset(warm[:, :], 0)
            nc.scalar.activation(
                out=scratch1,
                in_=const0,
                func=mybir.ActivationFunctionType.Identity,
                bias=const0,
            )

            # --- loads. Critical first: weights on the scalar HWDGE ring,
            # first row segment on the sync ring, then fine-grained row
            # segments alternating across both rings so row availability
            # stays ahead of the compute wavefront.
            nc.scalar.dma_start(out=wts, in_=w_d[:, :, :])
            nc.sync.dma_start(out=raw[:, 0:11, :], in_=x_bc[:, 0:11, :])
            nc.scalar.dma_start(
                out=bias_sb,
                in_=b_d.rearrange("b o -> (b o)").rearrange("(p x) -> p x", x=1),
            )
            nc.scalar.dma_start(out=raw[:, 11:19, :], in_=x_bc[:, 11:19, :])
            nc.sync.dma_start(out=raw[:, 19:35, :], in_=x_bc[:, 19:35, :])
            nc.scalar.dma_start(out=raw[:, 35:59, :], in_=x_bc[:, 35:59, :])
            nc.sync.dma_start(out=raw[:, 59:91, :], in_=x_bc[:, 59:91, :])
            nc.sync.dma_start(out=raw[:, 91:HP, :], in_=x_bc[:, 91:HP, :])

            # --- PE warm-up matmuls: zeros x zeros into a PSUM buffer that
            # the pool will recycle for the real accumulation later (the PE
            # queue is in-order, so the warm-up writes retire before the
            # first real matmul touches the same bank).
            warm_ps = psum.tile([128, BPC, QROWS, W], f32, name="ps0")
            for k in range(N_WARMUP_MM):
                nc.tensor.matmul(
                    warm_ps[:, k % BPC, :, :],
                    lhsT=warm[:, 0:128],
                    rhs=warm[:, :],
                    start=True,
                    stop=True,
                )

            y_bo = y_d.rearrange("b o h w -> (b o) h w")

            # --- main loop: pairs of spatial tiles, tap loop outer within
            # the pair so each stationary load serves G matmuls after
            # _dedup_ldweights. PSUM holds 2 pairs (8 banks) in flight.
            groups = (
                [[0]]
                + [[t, t + 1] for t in range(1, N_TILES - 1, G)]
                + [[N_TILES - 1]]
            )
            for tiles in groups:
                ps = {
                    t: psum.tile([128, BPC, QROWS, W], f32, name=f"ps{k}")
                    for k, t in enumerate(tiles)
                }
                for n, (i, j) in enumerate(TAPS):
                    for s in range(BPC):
                        for g in range(2):
                            for t in tiles:
                                r0 = t * TILE_ROWS + QROWS * g + i
                                nc.tensor.matmul(
                                    ps[t][64 * g : 64 * g + 64, s, :, :],
                                    lhsT=wts[64 * s : 64 * s + 64, i * KW + j, :],
                                    rhs=raw[
                                        64 * s : 64 * s + 64, r0 : r0 + QROWS, j : j + W
                                    ],
                                    start=(n == 0),
                                    stop=(n == len(TAPS) - 1),
                                )

                # --- evacuate each tile: per sample, the g=s quadrant is
                # partition-aligned with the output slot (DVE +bias), the
                # other crosses partitions (ACT +bias). Stores ride the idle
                # GpSimd SWDGE ring; the last tile stores as two 4-row
                # halves so the final transfer starts earlier.
                for t in tiles:
                    st = ostage.tile([128, TILE_ROWS, W], bf16, name=f"st{t % 4}")
                    last = t == N_TILES - 1
                    r = t * TILE_ROWS
                    for s in range(BPC):
                        home = slice(64 * s, 64 * s + 64)
                        away = slice(64 - 64 * s, 128 - 64 * s)
                        nc.vector.tensor_scalar_add(
                            st[home, QROWS * s : QROWS * s + QROWS, :],
                            ps[t][home, s, :, :],
                            bias_sb[home, :],
                        )
                        # ACT order flipped so that (DVE s) + (ACT 1-s)
                        # complete the st row-half s first for the split
                        # store below.
                        sa = 1 - s if last else s
                        homea = slice(64 * sa, 64 * sa + 64)
                        nc.scalar.activation(
                            out=st[homea, QROWS * (1 - sa) : QROWS * (2 - sa), :],
                            in_=ps[t][
                                slice(64 - 64 * sa, 128 - 64 * sa), sa, :, :
                            ],
                            func=mybir.ActivationFunctionType.Identity,
                            bias=bias_sb[homea, :],
                        )
                        if last:
                            # rows 4s:4s+4 of st are complete now
                            nc.gpsimd.dma_start(
                                out=y_bo[:, r + QROWS * s : r + QROWS * s + QROWS, :],
                                in_=st[:, QROWS * s : QROWS * s + QROWS, :],
                            )
                    if not last:
                        nc.gpsimd.dma_start(out=y_bo[:, r : r + TILE_ROWS, :], in_=st)
    return nc


def _get_module():
    if "nc" not in _CACHE:
        nc = _build_module()
        n = _dedup_ldweights(nc)
        assert n > 0, "expected redundant weight loads to drop"
        _split_multi_waits(nc)
        _CACHE["nc"] = nc
    return _CACHE["nc"]


def _in_maps(input, weight, bias):
    import ml_dtypes

    bf16 = ml_dtypes.bfloat16
    xpad = np.pad(input, ((0, 0), (0, 0), (1, 1), (1, 1)), mode="wrap").astype(bf16)
    maps = []
    for i in range(N_CORES):
        lo, hi = i * BPC, (i + 1) * BPC
        # wts[64b+c, tap, o] = w[b, c*9+tap, o]
        wloc = weight[lo:hi].reshape(BPC, C, KH * KW, O)
        maps.append(
            {
                "input": np.ascontiguousarray(xpad[lo:hi]),
                "wts": np.ascontiguousarray(wloc.reshape(BPC * C, KH * KW, O)).astype(
                    bf16
                ),
                "bias": np.ascontiguousarray(bias[lo:hi]),
            }
        )
    return maps


def kernel(input, weight, bias):
    from concourse.bass_utils import run_bass_kernel_spmd

    nc = _get_module()
    res = run_bass_kernel_spmd(
        nc, _in_maps(input, weight, bias), core_ids=list(range(N_CORES))
    )
    return np.concatenate(
        [res.results[i]["out"] for i in range(N_CORES)], axis=0
    ).astype(np.float32)
